# revision 1
# baseline (speedup 1.0000x reference)
"""DTW loss kernel for Trainium2 (Bass).

Computes sqrt(DTW^2(source, target)) for source, target of shape (2048,)
via the standard DP:
    D[i,j] = (s_i - t_j)^2 + min(D[i-1,j], D[i,j-1], D[i-1,j-1])

Mapping onto one NeuronCore (the problem is a single (source,target) pair, so
per the sharding hint there is no batch to parallelize over; all 8 cores run
the same program replicated and core 0's output is used):

- 128 column-chunks of 16 columns each; partition p owns columns [16p,16p+16).
- Wavefront: at step t partition p computes DP row r = t - 2*p (2 steps of
  slack per chunk so the boundary machinery stays off the critical path).
- One DP row-chunk = ONE vector-engine tensor_tensor_scan instruction:
  state = min(d0, state) + d1 over 32 interleaved slots (2 per cell):
    slot 2j   : d0 = D[r-1, j]   (up),      d1 = 0
    slot 2j+1 : d0 = D[r-1, j-1] (upleft),  d1 = c[r, j]
  so after slot 2j+1: state = c + min(up, upleft, left-carry)  == D[r, j].
  d0 is a 3-D access pattern (positions 2+2j-2i) over the previous row strip.
- Cross-chunk boundary (D[r, 16p-1]): PE matmul with a shifted-identity matrix
  moves each chunk's last column value to partition p+1 (PSUM), then the
  scalar engine copies it into the next strip's halo slot, adding a
  per-partition bias [1e30, 0, ...] to restore partition 0's boundary = INF.
- Costs c[r,j] are produced in bulk by the scalar engine (Square activation
  with per-partition bias -t_j) into a ring buffer from a host-prepared
  diagonally-shifted copy of source.
"""

import os
import sys

for _p in ("/opt/trn_rl_repo", "/root/.axon_site/_ro/trn_rl_repo"):
    if os.path.isdir(_p) and _p not in sys.path:
        sys.path.insert(0, _p)

import numpy as np

import concourse.bass as bass
import concourse.bacc as bacc
import concourse.mybir as mybir
import concourse.tile as tile
from concourse.bass_utils import run_bass_kernel_spmd

F32 = mybir.dt.float32

N = int(os.environ.get("DTW_N", "2048"))  # sequence length (both source and target)
P = 128             # partitions / column chunks
CW = N // P         # columns per chunk
SW = 2 * CW + 2     # strip width: [halo | 2*CW scan slots | pad]
SLACK = 2           # wavefront steps of slack per chunk
T = N + SLACK * (P - 1)   # 2302 total wavefront steps
M = T + 2                 # sdiag columns (padded)
RANGE = 256               # ring refill granularity (steps)
NRANGE = (T + RANGE - 1) // RANGE
RING = 3 * RANGE          # ring capacity in row-slots
INF = np.float32(1e30)
PAD = np.float32(1e15)    # sdiag pad; squares to 1e30

_cache = {}


def _build():
    nc = bacc.Bacc("TRN2", target_bir_lowering=False, debug=False)

    sdiag = nc.dram_tensor("sdiag", [P, M], F32, kind="ExternalInput")
    negt = nc.dram_tensor("negt", [P, CW], F32, kind="ExternalInput")
    shiftm = nc.dram_tensor("shiftm", [P, P], F32, kind="ExternalInput")
    biasfix = nc.dram_tensor("biasfix", [P, 1], F32, kind="ExternalInput")
    res = nc.dram_tensor("res", [1, 1], F32, kind="ExternalOutput")

    with tile.TileContext(nc) as tc:
        with (
            tc.tile_pool(name="sb", bufs=1) as pool,
            tc.tile_pool(name="ps", bufs=8, space="PSUM") as psp,
        ):
            t_sdiag = pool.tile([P, M], F32)
            t_negt = pool.tile([P, CW], F32)
            t_shift = pool.tile([P, P], F32)
            t_bias = pool.tile([P, 1], F32)
            t_ring = pool.tile([P, RING * 2 * CW], F32)
            t_sa = pool.tile([P, SW], F32)
            t_sb = pool.tile([P, SW], F32)

            nc.sync.dma_start(t_sdiag[:], sdiag[:])
            nc.sync.dma_start(t_negt[:], negt[:])
            nc.sync.dma_start(t_shift[:], shiftm[:])
            nc.sync.dma_start(t_bias[:], biasfix[:])

            # zeros in the even (d1) slots persist for the whole run
            nc.gpsimd.memset(t_ring[:], 0.0)
            nc.vector.memset(t_sa[:], float(INF))
            nc.vector.memset(t_sb[:], float(INF))
            # corner DTW[0,0] = 0 for the virtual row read by scan(0)
            nc.vector.memset(t_sb[0:1, 0:1], 0.0)

            strips = [t_sa, t_sb]
            eng = nc.vector

            def refill(g):
                if g >= NRANGE:
                    return
                m0 = g * RANGE
                cnt = min(RANGE, M - m0)
                base = (m0 % RING) * 2 * CW
                rstr = int(t_ring.ap[0][0])
                for j in range(CW):
                    out_ap = bass.AP(
                        t_ring.tensor,
                        t_ring.offset + base + 2 * j + 1,
                        [[rstr, P], [2 * CW, cnt]],
                    )
                    nc.scalar.activation(
                        out_ap,
                        t_sdiag[:, m0 : m0 + cnt],
                        mybir.ActivationFunctionType.Square,
                        bias=t_negt[:, j : j + 1],
                        scale=1.0,
                    )

            refill(0)
            refill(1)
            refill(2)

            pstr = int(t_sa.ap[0][0])
            for t in range(T):
                if t % RANGE == 0 and t > 0:
                    refill(t // RANGE + 2)
                cur = strips[t % 2]
                prev = strips[1 - (t % 2)]
                slot = (t % RING) * 2 * CW
                d0 = bass.AP(prev.tensor, prev.offset + 2, [[pstr, P], [2, CW], [-2, 2]])
                eng.add_instruction(
                    mybir.InstTensorScalarPtr(
                        name=nc.get_next_instruction_name(),
                        is_tensor_tensor_scan=True,
                        is_scalar_tensor_tensor=True,
                        op0=mybir.AluOpType.min,
                        op1=mybir.AluOpType.add,
                        ins=[
                            eng.lower_ap(d0),
                            eng.lower_ap(cur[:, 0:1]),
                            eng.lower_ap(t_ring[:, slot : slot + 2 * CW]),
                        ],
                        outs=[eng.lower_ap(cur[:, 1 : 2 * CW + 1])],
                    )
                )
                if t == 0:
                    # the 0.0 corner must be INF for every later read
                    nc.vector.memset(t_sb[0:1, 0:1], float(INF))
                if t >= 1:
                    # boundary machinery for step t-1 (consumed at t+1, t+2)
                    pcur = strips[(t - 1) % 2]
                    ps = psp.tile([P, 1], F32, tag="ps", name=f"ps{t}")
                    nc.tensor.matmul(ps[:], t_shift[:], pcur[:, 2 * CW : 2 * CW + 1])
                    nc.scalar.activation(
                        pcur[:, 0:1],
                        ps[:],
                        mybir.ActivationFunctionType.Identity,
                        bias=t_bias[:, 0:1],
                        scale=1.0,
                    )

            final = strips[(T - 1) % 2]
            nc.sync.dma_start(res[0:1, 0:1], final[P - 1 : P, 2 * CW : 2 * CW + 1])
    nc.compile()
    return nc


def _prep_inputs(source, target):
    source = np.asarray(source, np.float32).reshape(N)
    target = np.asarray(target, np.float32).reshape(N)
    sd = np.full((P, M), PAD, np.float32)
    for p in range(P):
        sd[p, SLACK * p : SLACK * p + N] = source
    negt = (-target.reshape(P, CW)).astype(np.float32)
    sh = np.zeros((P, P), np.float32)
    for p in range(1, P):
        sh[p - 1, p] = 1.0
    bf = np.zeros((P, 1), np.float32)
    bf[0, 0] = INF
    return {"sdiag": sd, "negt": negt, "shiftm": sh, "biasfix": bf}


def _run(inputs, trace=False):
    if "nc" not in _cache:
        _cache["nc"] = _build()
    nc = _cache["nc"]
    r = run_bass_kernel_spmd(
        nc, [dict(inputs) for _ in range(8)], core_ids=list(range(8)), trace=trace
    )
    return r


def kernel(source, target):
    inputs = _prep_inputs(source, target)
    r = _run(inputs)
    loss_sq = r.results[0]["res"][0, 0]
    return np.sqrt(np.float32(loss_sq))[None].astype(np.float32)



# revision 2
# speedup vs baseline: 1.6493x; 1.6493x over previous
"""DTW loss kernel for Trainium2 (Bass) — compact For_i wavefront version.

Computes sqrt(DTW^2(source, target)) for source, target of shape (2048,) via
    D[i,j] = (s_i - t_j)^2 + min(D[i-1,j], D[i,j-1], D[i-1,j-1])

Mapping (single NeuronCore; one (source,target) pair offers no batch
parallelism, so core 0 does all the work):

- 128 column-chunks of 16 columns each; partition p owns columns [16p,16p+16).
- Wavefront: at step t partition p computes DP row r = t - 2*p.
- One DP row-chunk = ONE vector-engine tensor_tensor_scan instruction:
  state = min(d0, state) + d1 over 32 interleaved slots (2 per cell).
- Cross-chunk boundary: PE matmul with a shifted-identity matrix moves each
  chunk's last column to partition p+1 (PSUM); scalar engine copies it into
  the next strip's halo slot, adding [1e30, 0, ...] to keep partition 0's
  boundary at INF.
- Costs are bulk-generated on the vector engine, 16 steps at a time, one
  iteration ahead of their use (write-ahead double duty of the cbuf tile).
- The t-loop runs as a hardware For_i with a 16-step unrolled body, so the
  whole program is ~250 instructions instead of ~9.5k. Per-call host
  dispatch cost (trace/lower/compile-cache hash) scales with program size,
  which is why this matters.
"""

import os
import sys

for _p in ("/opt/trn_rl_repo", "/root/.axon_site/_ro/trn_rl_repo"):
    if os.path.isdir(_p) and _p not in sys.path:
        sys.path.insert(0, _p)

import jax

jax.config.update("jax_compilation_cache_dir", "/tmp/jax_cc_cache")
jax.config.update("jax_persistent_cache_min_compile_time_secs", 0.0)
jax.config.update("jax_persistent_cache_min_entry_size_bytes", 0)

import numpy as np

import concourse.bass as bass
import concourse.bacc as bacc
import concourse.mybir as mybir
import concourse.tile as tile
from concourse.bass import ds
from concourse.bass_utils import run_bass_kernel_spmd

F32 = mybir.dt.float32

N = 2048            # sequence length (both source and target)
P = 128             # partitions / column chunks
CW = N // P         # 16 columns per chunk
SW = 2 * CW + 2     # strip width: [halo | 32 scan slots | pad]
SLACK = 2           # wavefront steps of slack per chunk
T = N + SLACK * (P - 1)   # 2302 total wavefront steps
B = 16              # steps per For_i iteration (body unroll)
TB0 = B             # first body iteration base (prologue covers 0..B-1)
TB1 = (T // B) * B  # 2288: loop covers [B, TB1); tail covers [TB1, T)
M = T + 2           # sdiag columns (covers cost prefetch to step T+1)
INF = np.float32(1e30)
PAD = np.float32(1e15)    # sdiag pad; squares to 1e30

_cache = {}


def _build(unroll=False):
    nc = bacc.Bacc("TRN2", target_bir_lowering=False, debug=False)

    sdiag = nc.dram_tensor("sdiag", [P, M], F32, kind="ExternalInput")
    negt = nc.dram_tensor("negt", [P, CW], F32, kind="ExternalInput")
    shiftm = nc.dram_tensor("shiftm", [P, P], F32, kind="ExternalInput")
    biasfix = nc.dram_tensor("biasfix", [P, 1], F32, kind="ExternalInput")
    res = nc.dram_tensor("res", [1, 1], F32, kind="ExternalOutput")

    with tile.TileContext(nc) as tc:
        with (
            tc.tile_pool(name="sb", bufs=1) as pool,
            tc.tile_pool(name="ps", bufs=8, space="PSUM") as psp,
        ):
            t_sdiag = pool.tile([P, M], F32)
            t_negt = pool.tile([P, CW], F32)
            t_shift = pool.tile([P, P], F32)
            t_bias = pool.tile([P, 1], F32)
            t_cb = pool.tile([P, B * 2 * CW], F32)
            t_sa = pool.tile([P, SW], F32)
            t_sb = pool.tile([P, SW], F32)
            t_res = pool.tile([P, 1], F32)

            nc.sync.dma_start(t_sdiag[:], sdiag[:])
            nc.sync.dma_start(t_negt[:], negt[:])
            nc.sync.dma_start(t_shift[:], shiftm[:])
            nc.sync.dma_start(t_bias[:], biasfix[:])

            # zeros in the even (d1) slots persist for the whole run
            nc.gpsimd.memset(t_cb[:], 0.0)
            nc.vector.memset(t_sa[:], float(INF))
            nc.vector.memset(t_sb[:], float(INF))
            # corner DTW[0,0] = 0 for the virtual row read by scan(0)
            nc.vector.memset(t_sb[0:1, 0:1], 0.0)

            strips = [t_sa, t_sb]
            eng = nc.vector
            pstr = int(t_sa.ap[0][0])
            sdw = int(t_sdiag.ap[0][0])
            nw = int(t_negt.ap[0][0])
            cbw = int(t_cb.ap[0][0])

            def costgen(base_off):
                """Fill cbuf odd slots with costs for steps base..base+B-1.

                base_off: int (static) or RuntimeValue (dynamic) element
                offset into sdiag. Two DVE tensor_tensor ops:
                  cb[p, k*32 + 2j+1] = (sdiag[p, base+k] - negt[p, j])^2
                """
                in0 = bass.AP(
                    t_sdiag.tensor, base_off + t_sdiag.offset,
                    [[sdw, P], [1, B], [0, CW]],
                )
                in1 = bass.AP(
                    t_negt.tensor, t_negt.offset, [[nw, P], [0, B], [1, CW]]
                )
                out0 = bass.AP(
                    t_cb.tensor, t_cb.offset + 1, [[cbw, P], [2 * CW, B], [2, CW]]
                )
                # negt holds -target, so add gives (s - t); then square in place
                nc.vector.tensor_tensor(out0, in0, in1, mybir.AluOpType.add)
                nc.vector.tensor_tensor(out0, out0, out0, mybir.AluOpType.mult)

            def scan(k):
                """One DP row-chunk step at body position k (t = tb + k)."""
                cur = strips[k % 2]
                prev = strips[1 - (k % 2)]
                d0 = bass.AP(
                    prev.tensor, prev.offset + 2, [[pstr, P], [2, CW], [-2, 2]]
                )
                eng.add_instruction(
                    mybir.InstTensorScalarPtr(
                        name=nc.get_next_instruction_name(),
                        is_tensor_tensor_scan=True,
                        is_scalar_tensor_tensor=True,
                        op0=mybir.AluOpType.min,
                        op1=mybir.AluOpType.add,
                        ins=[
                            eng.lower_ap(d0),
                            eng.lower_ap(cur[:, 0:1]),
                            eng.lower_ap(t_cb[:, k * 2 * CW : (k + 1) * 2 * CW]),
                        ],
                        outs=[eng.lower_ap(cur[:, 1 : 2 * CW + 1])],
                    )
                )

            def machinery(k, name):
                """Boundary propagation for step t-1 (emitted at position k)."""
                pcur = strips[(k - 1) % 2]
                ps = psp.tile([P, 1], F32, tag="ps", name=name)
                nc.tensor.matmul(ps[:], t_shift[:], pcur[:, 2 * CW : 2 * CW + 1])
                nc.scalar.activation(
                    pcur[:, 0:1],
                    ps[:],
                    mybir.ActivationFunctionType.Identity,
                    bias=t_bias[:, 0:1],
                    scale=1.0,
                )

            # ---- prologue: steps 0..B-1 (static) ----
            costgen(0)
            for k in range(B):
                scan(k)
                if k == 0:
                    # the 0.0 corner must be INF for every later read
                    nc.vector.memset(t_sb[0:1, 0:1], float(INF))
                else:
                    machinery(k, f"pp{k}")
            costgen(TB0)  # prefill costs for the first body iteration

            # ---- body: steps TB0..TB1-1 via hardware loop ----
            if unroll:
                for tb in range(TB0, TB1, B):
                    for k in range(B):
                        scan(k)
                        machinery(k, f"pu{tb}_{k}")
                    costgen(tb + B)
            else:
                with tc.For_i(TB0, TB1, B) as tb:
                    for k in range(B):
                        scan(k)
                        machinery(k, f"pb{k}")
                    costgen(tb + B)

            # ---- tail: steps TB1..T-1 (static) ----
            for k in range(T - TB1):
                scan(k)
                machinery(k, f"pt{k}")

            # ---- result: D[N-1 cols...] at strips[(T-1)%2][P-1, 2*CW] ----
            final = strips[(T - 1) % 2]
            nc.scalar.activation(
                t_res[:, 0:1],
                final[:, 2 * CW : 2 * CW + 1],
                mybir.ActivationFunctionType.Sqrt,
            )
            nc.sync.dma_start(res[0:1, 0:1], t_res[P - 1 : P, 0:1])
    nc.compile()
    return nc


def _prep_inputs(source, target):
    source = np.asarray(source, np.float32).reshape(N)
    target = np.asarray(target, np.float32).reshape(N)
    sd = np.full((P, M), PAD, np.float32)
    for p in range(P):
        sd[p, SLACK * p : SLACK * p + N] = source
    negt = (-target.reshape(P, CW)).astype(np.float32)
    sh = np.zeros((P, P), np.float32)
    for p in range(1, P):
        sh[p - 1, p] = 1.0
    bf = np.zeros((P, 1), np.float32)
    bf[0, 0] = INF
    return {"sdiag": sd, "negt": negt, "shiftm": sh, "biasfix": bf}


def _run(inputs, trace=False):
    if "nc" not in _cache:
        _cache["nc"] = _build()
    nc = _cache["nc"]
    r = run_bass_kernel_spmd(nc, [dict(inputs)], core_ids=[0], trace=trace)
    return r


def kernel(source, target):
    inputs = _prep_inputs(source, target)
    r = _run(inputs)
    return r.results[0]["res"].reshape(1).astype(np.float32)
